# revision 9
# baseline (speedup 1.0000x reference)
"""MetaUpsampler Trainium2 kernel (8-core SPMD, full-I/O contract).

Phase-grouped formulation: output pixel (oy, ox) = (2t+p, 2u+r), scale=2.
Per core: 16 consecutive t-rows x all 4 phases x both batches.

Device pipeline (feature-major, im2col-free):
  z1   = sum_i lhs1_i^T @ V_i          (5 accumulated matmuls over shifted-AP
                                        views of 3 clamp-corrected x-shifted
                                        copies of the padded feature map)
  a1   = gelu(z1 + mc[phase])          (meta-MLP folded into per-phase bias)
  lgt  = wk2t^T @ a1                   (packed 4 phases per PSUM tile)
  E    = exp(lgt + bk2)
  Z    = zpat^T @ E ; rZ = 1/Z ; rZb = brep^T @ rZ ; Et = E * rZb
  per chunk i: Eb = rep_i^T @ Et[9 rows] ; P = V_i * Eb
  h    = gelu(sum_i sup_i^T @ P + br1) (fold matmul fuses softmax-weighted
                                        patch sum with rgb layer 1; absorbs
                                        the torch-style misaligned reshape)
  rgb  = wr2t^T @ h + br2              (packed 4 phases, biased copy, DMA out)
Host interleaves the 4 phase grids into (B, 3, 256, 256).
"""

import math
import sys
import time

import numpy as np

if "/opt/trn_rl_repo" not in sys.path:
    sys.path.insert(0, "/opt/trn_rl_repo")

C = 64
K2 = 9
BANDS = 8
H = W = 128
NCORES = 8
TPC = H // NCORES  # t-rows per core (16)
XW = 129  # x-columns in the shifted feature arrays (xx = u + r in [0, 128])
# kappa order: chunk i holds rows (c under SIGMA[2i]) then (c under SIGMA[2i+1])
SIGMA = [(0, 0), (0, 1), (1, 0), (1, 1), (2, 0), (2, 1), (0, 2), (1, 2), (2, 2)]
# (tile-kind, kh-lower) per chunk: 0..2 -> fpad2 (fL;fM), 3,4 -> fpad3 (fR;fR+y)
CHUNK_TILE = [(2, 0), (2, 1), (2, 2), (3, 0), (3, 2)]
# t-groups (relative t, length); first group isolated so the t=0 row clamp
# (core 0) can use the separate fp_t0 tensors with a core-uniform program.
TGROUPS = [(0, 1), (1, 3), (4, 3), (7, 3), (10, 3), (13, 3)]


def _gelu_np(x):
    from scipy.special import erf

    return (x * 0.5 * (1.0 + erf(x / np.sqrt(2.0)))).astype(np.float32)


def host_prep(feat, w1m, b1m, w2m, b2m, wk1, bk1, wk2, bk2, wr1, br1, wr2,
              br2, scale):
    """All static/host-side preparation. Returns (consts, per-core maps, B)."""
    feat = np.asarray(feat, dtype=np.float32)
    B = feat.shape[0]
    s = float(int(scale))
    assert s == 2.0 and feat.shape[1] == C and feat.shape[2] == H

    # ---- meta branch (4 phase variants; fp32 host math) ----
    kappa = max(0.1, 1.0 / s)
    eta = min(1.0, 0.15 * s)
    freqs = (2.0 ** np.arange(BANDS, dtype=np.float32)) * np.float32(math.pi)
    mc = np.zeros((4, 128), dtype=np.float32)  # phase ph = 2*p + r
    for p in (0, 1):
        dv = np.float32(0.25 if p == 0 else -0.25)
        for r in (0, 1):
            du = np.float32(0.25 if r == 0 else -0.25)
            m = np.array([s, du, dv, kappa, eta], dtype=np.float32)
            xb = (m[:, None] * freqs[None, :]).astype(np.float32)
            enc = np.concatenate(
                [m[:, None], np.sin(xb), np.cos(xb)], axis=1
            ).astype(np.float32).reshape(-1)
            h1 = _gelu_np((enc @ w1m.T + b1m).astype(np.float32))
            m_emb = (h1 @ w2m.T + b2m).astype(np.float32)
            mc[2 * p + r] = (wk1[:, C * K2:] @ m_emb + bk1).astype(np.float32)
    mc_t = np.ascontiguousarray(mc.T)  # [128, 4]

    # ---- padded feature and three clamp-corrected x-shifted copies ----
    # Pr coords: np.pad output, rows/cols in [0, 130). Patch read (pixel
    # (p,r,t,u), offset (kh,kw)) = Pr[cy+kh, cx+kw], cy/cx = clip(.-1+., 0, 127)
    fpad = np.pad(feat, ((0, 0), (0, 0), (1, 1), (1, 1)), mode="reflect")

    def xarr(s):
        # col xx = u + r; value Pr[.., xx - 1 + s] generically; col 0 carries
        # the u=0,r=0 clamp correction Pr[.., s] (== col 1, since cx clamps).
        a = np.empty((B, C, H + 2, XW), dtype=np.float32)
        a[..., 1:] = fpad[..., s: s + 128]
        a[..., 0] = fpad[..., s]
        return a

    fx = {"fl": xarr(0), "fm": xarr(1), "fr": xarr(2)}  # [B, C, 130y, 129x]

    # ---- static matrices ----
    lhs1 = np.zeros((5, 128, 128), dtype=np.float32)
    jorig = np.zeros((5, 128), dtype=np.int64)
    for i in range(5):
        for half in range(2 if i < 4 else 1):
            kh, kw = SIGMA[2 * i + half]
            for c in range(C):
                row = half * C + c
                j = c * K2 + kh * 3 + kw
                jorig[i, row] = j
                lhs1[i, row, :] = wk1[:, j]
    rep = np.zeros((5, 128, 128), dtype=np.float32)
    sup = np.zeros((5, 128, C), dtype=np.float32)
    for i in range(5):
        nrows = 128 if i < 4 else C
        for row in range(nrows):
            j = int(jorig[i, row])
            for blk in range(4):
                rep[i, 32 * blk + j // C, row] = 1.0
            sup[i, row, :] = wr1[:, j % C]

    wk2t = np.zeros((128, 32), dtype=np.float32)
    wk2t[:, :K2] = wk2.T
    bk2pack = np.zeros((128, 1), dtype=np.float32)
    zpat = np.zeros((128, 4), dtype=np.float32)
    brep = np.zeros((4, 128), dtype=np.float32)
    br2pack = np.zeros((128, 1), dtype=np.float32)
    for blk in range(4):
        bk2pack[32 * blk: 32 * blk + K2, 0] = bk2
        zpat[32 * blk: 32 * blk + K2, blk] = 1.0
        brep[blk, 32 * blk: 32 * blk + K2] = 1.0
        br2pack[32 * blk: 32 * blk + 3, 0] = br2
    consts = dict(
        lhs1=np.ascontiguousarray(lhs1),
        rep=np.ascontiguousarray(rep),
        sup=np.ascontiguousarray(sup),
        wk2t=wk2t, bk2pack=bk2pack, zpat=zpat, brep=brep, br2pack=br2pack,
        wr2t=np.ascontiguousarray(wr2.T.astype(np.float32)),
        br1c=np.ascontiguousarray(br1.reshape(C, 1).astype(np.float32)),
        mc=mc_t,
    )

    # ---- per-core input slices ----
    in_maps = []
    for k in range(NCORES):
        t0 = k * TPC
        y0 = t0 - 1  # main-buffer row g holds padded row (y0 + g), g in [0,19)
        m = dict(consts)
        for nm, arr in fx.items():
            # main buffer row g holds Pr row (y0 + g); rows outside [0, 130)
            # are zero (only g = 0 on core 0, never read: tg >= 1 => g >= 1)
            sl = np.zeros((B, C, 19, XW), dtype=np.float32)
            gs = max(0, -y0)
            ge = min(19, 130 - y0)
            sl[:, :, gs:ge] = arr[:, :, y0 + gs: y0 + ge, :]
            m[nm] = sl
            # fp_t0 row (3p + kh) holds Pr row clip(t0-1+p, 0, 127) + kh
            t0rows = np.zeros((B, C, 6, XW), dtype=np.float32)
            for p in (0, 1):
                base = min(max(t0 - 1 + p, 0), 127)
                for kh in range(3):
                    t0rows[:, :, 3 * p + kh] = arr[:, :, base + kh, :]
            m["t_" + nm] = np.ascontiguousarray(t0rows)
        in_maps.append(m)
    return consts, in_maps, B


def build(B, reps=1):
    import concourse.bacc as bacc
    import concourse.mybir as mybir
    from concourse import tile

    fp32 = mybir.dt.float32
    AF = mybir.ActivationFunctionType

    nc = bacc.Bacc("TRN2", target_bir_lowering=False, debug=False)

    d_f = {nm: nc.dram_tensor(nm, [B, C, 19, XW], fp32, kind="ExternalInput")
           for nm in ("fl", "fm", "fr")}
    d_t = {nm: nc.dram_tensor("t_" + nm, [B, C, 6, XW], fp32,
                              kind="ExternalInput")
           for nm in ("fl", "fm", "fr")}
    d_lhs1 = nc.dram_tensor("lhs1", [5, 128, 128], fp32, kind="ExternalInput")
    d_rep = nc.dram_tensor("rep", [5, 128, 128], fp32, kind="ExternalInput")
    d_sup = nc.dram_tensor("sup", [5, 128, C], fp32, kind="ExternalInput")
    d_wk2t = nc.dram_tensor("wk2t", [128, 32], fp32, kind="ExternalInput")
    d_bk2 = nc.dram_tensor("bk2pack", [128, 1], fp32, kind="ExternalInput")
    d_zpat = nc.dram_tensor("zpat", [128, 4], fp32, kind="ExternalInput")
    d_brep = nc.dram_tensor("brep", [4, 128], fp32, kind="ExternalInput")
    d_br2 = nc.dram_tensor("br2pack", [128, 1], fp32, kind="ExternalInput")
    d_wr2t = nc.dram_tensor("wr2t", [C, 3], fp32, kind="ExternalInput")
    d_br1 = nc.dram_tensor("br1c", [C, 1], fp32, kind="ExternalInput")
    d_mc = nc.dram_tensor("mc", [128, 4], fp32, kind="ExternalInput")
    d_out = nc.dram_tensor("out", [B, 2, 2, 3, TPC, 128], fp32,
                           kind="ExternalOutput")

    with tile.TileContext(nc) as tc:
        with (
            tc.tile_pool(name="fp", bufs=1) as fpp,
            tc.tile_pool(name="cst", bufs=1) as cst,
            tc.tile_pool(name="z1p", bufs=2, space="PSUM") as z1p,
            tc.tile_pool(name="lgp", bufs=2, space="PSUM") as lgp,
            tc.tile_pool(name="mscp", bufs=2, space="PSUM") as mscp,
            tc.tile_pool(name="ebp", bufs=1, space="PSUM") as ebp,
            tc.tile_pool(name="hpp", bufs=1, space="PSUM") as hpp,
            tc.tile_pool(name="sbp", bufs=3) as sbp,
        ):
            # ---- constants into SBUF ----
            def cload(dram, shape, tag):
                t = cst.tile(shape, fp32, tag=tag)
                nc.sync.dma_start(out=t[:], in_=dram[:])
                return t

            c_lhs1, c_rep, c_sup = [], [], []
            for i in range(5):
                t = cst.tile([128, 128], fp32, tag=f"lhs1_{i}")
                nc.sync.dma_start(out=t[:], in_=d_lhs1[i])
                c_lhs1.append(t)
                t = cst.tile([128, 128], fp32, tag=f"rep_{i}")
                nc.sync.dma_start(out=t[:], in_=d_rep[i])
                c_rep.append(t)
                t = cst.tile([128, C], fp32, tag=f"sup_{i}")
                nc.sync.dma_start(out=t[:], in_=d_sup[i])
                c_sup.append(t)
            c_wk2t = cload(d_wk2t, [128, 32], "wk2t")
            c_bk2 = cload(d_bk2, [128, 1], "bk2")
            c_zpat = cload(d_zpat, [128, 4], "zpat")
            c_brep = cload(d_brep, [4, 128], "brep")
            c_br2 = cload(d_br2, [128, 1], "br2")
            c_wr2t = cload(d_wr2t, [C, 3], "wr2t")
            c_br1 = cload(d_br1, [C, 1], "br1")
            c_mc = cload(d_mc, [128, 4], "mc")

            # ---- feature map tiles ----
            # fpad2[b] = [fL ; fM], fpad3[b] = [fR ; fR shifted y+1]
            # fpt2[b] = [tL ; tM],  fpt3[b] = [tR ; tR shifted row+1]
            YW = 19 * XW
            TW = 6 * XW
            fpad2, fpad3, fpt2, fpt3 = [], [], [], []
            for b in range(B):
                f2 = fpp.tile([128, YW], fp32, tag=f"f2_{b}")
                nc.sync.dma_start(out=f2[0:C, :],
                                  in_=d_f["fl"][b].rearrange("c y x -> c (y x)"))
                nc.sync.dma_start(out=f2[C:128, :],
                                  in_=d_f["fm"][b].rearrange("c y x -> c (y x)"))
                f3 = fpp.tile([128, YW], fp32, tag=f"f3_{b}")
                nc.sync.dma_start(out=f3[0:C, :],
                                  in_=d_f["fr"][b].rearrange("c y x -> c (y x)"))
                nc.sync.dma_start(
                    out=f3[C:128, 0:18 * XW],
                    in_=d_f["fr"][b, :, 1:19, :].rearrange("c y x -> c (y x)"))
                t2 = fpp.tile([128, TW], fp32, tag=f"t2_{b}")
                nc.sync.dma_start(out=t2[0:C, :],
                                  in_=d_t["fl"][b].rearrange("c y x -> c (y x)"))
                nc.sync.dma_start(out=t2[C:128, :],
                                  in_=d_t["fm"][b].rearrange("c y x -> c (y x)"))
                t3 = fpp.tile([128, TW], fp32, tag=f"t3_{b}")
                nc.sync.dma_start(out=t3[0:C, :],
                                  in_=d_t["fr"][b].rearrange("c y x -> c (y x)"))
                nc.sync.dma_start(
                    out=t3[C:128, 0:5 * XW],
                    in_=d_t["fr"][b, :, 1:6, :].rearrange("c y x -> c (y x)"))
                fpad2.append(f2)
                fpad3.append(f3)
                fpt2.append(t2)
                fpt3.append(t3)

            import concourse.bass as bass

            def vap2(b, p, r, tg, L, i):
                """AP view of patch-chunk i for item (b,p,r,tg,L)."""
                kind, khl = CHUNK_TILE[i]
                if tg == 0:
                    tl = (fpt2[b] if kind == 2 else fpt3[b])
                    off = (3 * p + khl) * XW + r
                    return tl[:, off: off + 128].unsqueeze(1)
                tl = (fpad2[b] if kind == 2 else fpad3[b])
                g0 = tg + p + khl  # buffer row of padded row R(tg)+khl
                base = g0 * XW + r
                full = tl[:, base: base + (L - 1) * XW + 128]
                if L == 1:
                    return full.unsqueeze(1)
                # overlapping windows: [t: L, step XW], [u: 128, step 1]
                return bass.AP(full.tensor, full.offset,
                               [list(full.ap[0]), [XW, L], [1, 128]])

            for _rep in range(reps):
              for b in range(B):
                for (tg, L) in TGROUPS:
                    n = L * 128
                    lg = lgp.tile([128, n], fp32, tag="lg")
                    a1s = {}
                    for p in (0, 1):
                        for r in (0, 1):
                            ph = 2 * p + r
                            z1 = z1p.tile([128, n], fp32, tag="z1")
                            for i in range(5):
                                nc.tensor.matmul(
                                    z1[:], c_lhs1[i][:], vap2(b, p, r, tg, L, i),
                                    start=(i == 0), stop=(i == 4))
                            a1 = sbp.tile([128, n], fp32, tag="a1")
                            nc.scalar.activation(
                                a1[:], z1[:], AF.Gelu,
                                bias=c_mc[:, ph:ph + 1])
                            a1s[ph] = a1
                            nc.tensor.matmul(
                                lg[32 * ph: 32 * ph + 32, :], c_wk2t[:],
                                a1[:], start=True, stop=True,
                                tile_position=(0, 32 * ph))
                    E = sbp.tile([128, n], fp32, tag="E")
                    nc.scalar.activation(E[:], lg[:], AF.Exp, bias=c_bk2[:, 0:1])
                    Zp = mscp.tile([4, n], fp32, tag="msc")
                    nc.tensor.matmul(Zp[:], c_zpat[:], E[:], start=True,
                                     stop=True)
                    rZ = sbp.tile([4, n], fp32, tag="rZ")
                    nc.vector.reciprocal(rZ[:], Zp[:])
                    rZb = mscp.tile([128, n], fp32, tag="msc")
                    nc.tensor.matmul(rZb[:], c_brep[:], rZ[:], start=True,
                                     stop=True)
                    Et = sbp.tile([128, n], fp32, tag="Et")
                    nc.vector.tensor_mul(Et[:], E[:], rZb[:])

                    rgbp = lgp.tile([128, n], fp32, tag="lg")
                    for p in (0, 1):
                        for r in (0, 1):
                            ph = 2 * p + r
                            hps = hpp.tile([C, n], fp32, tag="hps")
                            for i in range(5):
                                eb = ebp.tile([128, n], fp32, tag="eb")
                                nc.tensor.matmul(
                                    eb[:],
                                    c_rep[i][32 * ph: 32 * ph + 9, :],
                                    Et[32 * ph: 32 * ph + 9, :],
                                    start=True, stop=True,
                                    tile_position=(32 * ph, 0))
                                P = sbp.tile([128, n], fp32, tag="P")
                                nc.vector.tensor_mul(
                                    P[:], vap2(b, p, r, tg, L, i), eb[:])
                                nc.tensor.matmul(
                                    hps[:], c_sup[i][:], P[:],
                                    start=(i == 0), stop=(i == 4))
                            hsb = sbp.tile([C, n], fp32, tag="h")
                            nc.scalar.activation(hsb[:], hps[:], AF.Gelu,
                                                 bias=c_br1[:, 0:1])
                            nc.tensor.matmul(
                                rgbp[32 * ph: 32 * ph + 3, :], c_wr2t[:],
                                hsb[:], start=True, stop=True,
                                tile_position=(0, 32 * ph))
                    rgbs = sbp.tile([128, n], fp32, tag="rgbs")
                    nc.vector.tensor_scalar_add(rgbs[:], rgbp[:],
                                                c_br2[:, 0:1])
                    for p in (0, 1):
                        for r in (0, 1):
                            ph = 2 * p + r
                            src = rgbs[32 * ph: 32 * ph + 3, :].rearrange(
                                "c (t u) -> c t u", t=L)
                            nc.sync.dma_start(
                                out=d_out[b, p, r, :, tg: tg + L, :], in_=src)
    nc.compile()
    return nc


_CACHE = {}


def kernel(**inputs):
    from concourse.bass_utils import run_bass_kernel_spmd

    consts, in_maps, B = host_prep(**inputs)
    key = ("nc", B)
    if key not in _CACHE:
        _CACHE[key] = build(B)
    nc = _CACHE[key]
    res = run_bass_kernel_spmd(nc, in_maps, list(range(NCORES)))
    out = np.zeros((B, 3, 2 * H, 2 * W), dtype=np.float32)
    for k in range(NCORES):
        o = res.results[k]["out"]  # [B, 2, 2, 3, TPC, 128]
        t0 = k * TPC
        for p in (0, 1):
            for r in (0, 1):
                out[:, :, 2 * t0 + p: 2 * (t0 + TPC) + p: 2, r::2] = \
                    o[:, p, r]
    return out
